# revision 1
# baseline (speedup 1.0000x reference)
"""DepthAttentionResidual Trainium2 kernel.

Computation (see reference):
    ms      = mean(history^2, axis=-1)                      # [S,B,T]
    logits  = dot(query*rms_weight, history) * rsqrt(ms+eps)
    w       = softmax(logits, axis=S)
    out     = sum_s w[s] * history[s]                        # [B,T,D]

Sharding: data-parallel over (B=4) x (T halves) = 8 cores. Each core gets
hist [S=16, Tc=1024, D=1024] (64 MiB) and produces out [1024, 1024].

Per-core layout: partition p = s*8 + t' (S=16 depths x 8 t-blocks), D on
the free axis. A supertile is 128 t; slice g holds t_local = t'*16 + g,
so one slice is [128 partitions, 1024].

Bottleneck model (HW-measured): with all 8 cores running, the two
NeuronCores of each SEngine share the 16 SBUF AXI ports (435 GB/s), so
each core's SBUF-side DMA traffic caps at ~218 GB/s regardless of
descriptor size. The fp32 stream is therefore converted to fp16 *during
the DMA* (SWDGE cast on the gpsimd queue): HBM read stays 64 MiB but
SBUF writes drop to 32 MiB. Masks, the mix weights, the matmuls and the
output stream are fp16 as well (output upcast to fp32 on the host).
After that the kernel is compute-bound: ScalarE (Square-sumsq) and
VectorE (dot reduces + w2 builds) both run ~90% busy at ~26us per
supertile vs the ~21us DMA pace.
  - sum(h^2) over D: ScalarE activation(Square, accum_out); last slice
    on VectorE to balance engines
  - dot(q*w, h) over D: VectorE affine_mul_reduce (tensor_tensor_reduce
    crashes this runtime; scalar_tensor_tensor is 2x slower on DVE and
    ISA-illegal on GpSimd)
  - softmax over S: sum over s-partition-subgroups via 0/1-mask fp32
    matmuls (exact); exp on ScalarE; reciprocals on VectorE
  - depth mix: per D-half, 16 accumulating fp16 matmuls with
    block-expanded masked weights built on VectorE (w2[p, c] = softmax
    weight iff c == t_local(p, g)); PSUM -> SBUF on ScalarE (activation
    Copy); one DMA per supertile on the ScalarE HWDGE ring.
GpSimd carries NOTHING but the SWDGE dma_starts: any compute on the
in-order Pool queue (even dependency-free, even interleaved between
dma_starts) starves the descriptor ring and gaps the history stream —
measured +40-70us on every variant that tried.
The last supertile is split 64/64 to shorten the serial tail. fp16
operands -> ~7e-4 relative output error (gate is 2e-2).
"""
import numpy as np

import concourse.bass as bass
import concourse.bacc as bacc
import concourse.tile as tile
from concourse import mybir
from concourse import bass_utils

N_CORES = 8
S = 16
B = 4
T = 2048
D = 1024
EPS = 1e-5

TC = T // 2          # t positions per core
TG = 8               # t-blocks per partition set (S * TG = 128 partitions)
GROUPS = 16          # stat slices per supertile (one t per partition each)
J = 2                # consecutive t rows per DMA slice (read desc = J*4KiB)
TS = TG * GROUPS     # t per supertile = 128
N_SUPER = TC // TS   # supertiles per core = 8
F32 = mybir.dt.float32
F16 = mybir.dt.float16


def _build_program():
    nc = bacc.Bacc("TRN2", target_bir_lowering=False, debug=False,
                   enable_asserts=True, num_devices=N_CORES)

    hist = nc.dram_tensor("hist", [S, TC, D], F32, kind="ExternalInput").ap()
    query = nc.dram_tensor("query", [D], F32, kind="ExternalInput").ap()
    rmsw = nc.dram_tensor("rms_weight", [D], F32, kind="ExternalInput").ap()
    mask_d = nc.dram_tensor("mask", [128, TG], F32, kind="ExternalInput").ap()
    maskt_d = nc.dram_tensor("maskT", [TG, 128], F32, kind="ExternalInput").ap()
    maskf_d = nc.dram_tensor("maskF", [128, GROUPS, 128], F16,
                             kind="ExternalInput").ap()
    maskf8_d = nc.dram_tensor("maskF8", [128, GROUPS // 2, 128 // 2], F16,
                              kind="ExternalInput").ap()
    out = nc.dram_tensor("out", [TC, D], F16, kind="ExternalOutput").ap()

    with tile.TileContext(nc) as tc:
        with (
            tc.tile_pool(name="singles", bufs=1) as singles,
            tc.tile_pool(name="hsup", bufs=2) as hpool,
            tc.tile_pool(name="stats", bufs=2) as stats,
            tc.tile_pool(name="w2", bufs=3) as w2pool,
            tc.tile_pool(name="outp", bufs=2) as outpool,
            tc.tile_pool(name="ps_stats", bufs=2, space="PSUM") as ps_stats,
            tc.tile_pool(name="ps_mix", bufs=2, space="PSUM") as ps_mix,
        ):
            # ---- constants ------------------------------------------------
            qf = singles.tile([128, D], F32)
            wf = singles.tile([128, D], F32)
            q16 = singles.tile([128, D], F16)
            mask = singles.tile([128, TG], F32)
            maskT = singles.tile([TG, 128], F32)
            maskF = singles.tile([128, GROUPS, 128], F16)
            maskF8 = singles.tile([128, GROUPS // 2, 128 // 2], F16)
            epst = singles.tile([128, 1], F32)
            dummy_a = singles.tile([128, 1], F32)
            dummy_v = singles.tile([128, 1], F32)

            def emit_init():
                # constants ride the ScalarE HWDGE ring so the SWDGE
                # (gpsimd) queue carries only the history stream and its
                # first dma_start issues immediately
                nc.scalar.dma_start(
                    out=qf[:],
                    in_=bass.AP(tensor=query.tensor, offset=0,
                                ap=[[0, 128], [1, D]]),
                )
                nc.scalar.dma_start(
                    out=wf[:],
                    in_=bass.AP(tensor=rmsw.tensor, offset=0,
                                ap=[[0, 128], [1, D]]),
                )
                # query * rms_weight, cast to fp16 in the same DVE op
                nc.vector.tensor_mul(q16[:], qf[:], wf[:])
                nc.scalar.dma_start(out=mask[:], in_=mask_d)
                nc.scalar.dma_start(out=maskT[:], in_=maskt_d)
                nc.scalar.dma_start(out=maskF[:], in_=maskf_d)
                nc.scalar.dma_start(out=maskF8[:], in_=maskf8_d)
                nc.vector.memset(epst[:], EPS)

            # ---- main loop over supertiles --------------------------------
            # last 128-t supertile is split in two 64-t halves to shorten
            # the serial tail after the final DMA
            emit_init()
            schedule = [(k * TS, GROUPS) for k in range(N_SUPER - 1)]
            schedule += [((N_SUPER - 1) * TS, GROUPS // 2),
                         ((N_SUPER - 1) * TS + TS // 2, GROUPS // 2)]
            for k, (t0, groups) in enumerate(schedule):
                ts_k = TG * groups
                ndma = groups // J

                # load [S, 128t, D] as partitions (s, t') x free (g, d)
                # where t_local = t' * groups + g; fp32->fp16 cast during
                # the DMA (read desc J*4KiB, SBUF write desc J*2KiB)
                half = max(ndma // 2, 1)
                hsupA = hpool.tile([128, half, J, D], F16, tag="hsupA",
                                   name="hsupA", bufs=5)
                hsupB = (hpool.tile([128, ndma - half, J, D], F16,
                                    tag="hsupB", name="hsupB", bufs=5)
                         if ndma > half else None)

                def hslice(g, hsupA=hsupA, hsupB=hsupB, half=half):
                    gd, j = g // J, g % J
                    tile_ = hsupA if gd < half else hsupB
                    return tile_[:, gd - half if gd >= half else gd, j, :]

                srcv = hist[:, t0:t0 + ts_k, :].rearrange(
                    "s (t gd j) d -> s t gd (j d)", t=TG, gd=ndma)
                for gd in range(ndma):
                    tile_ = hsupA if gd < half else hsupB
                    nc.gpsimd.dma_start(
                        out=tile_[:, gd - half if gd >= half else gd, :, :]
                        .rearrange("p j d -> p (j d)"),
                        in_=srcv[:, :, gd, :])

                ss = stats.tile([128, groups], F32, tag="ss")
                dot = stats.tile([128, groups], F32, tag="dot")
                # engine split for the 2*groups D-reductions: dots all on
                # VectorE (affine_mul_reduce); sumsq mostly on ScalarE
                n_act = (15 * groups) // GROUPS
                for g in range(groups):
                    h_g = hslice(g)
                    if g < n_act:
                        nc.scalar.activation(
                            out=dummy_a.broadcast_to([128, D]),
                            in_=h_g,
                            func=mybir.ActivationFunctionType.Square,
                            accum_out=ss[:, g:g + 1],
                        )
                    else:
                        nc.vector.affine_mul_reduce(
                            out=dummy_v.broadcast_to([128, D]),
                            accum_out=ss[:, g:g + 1],
                            in0=h_g, in1=h_g, scale=1.0, bias=0.0,
                        )
                    nc.vector.affine_mul_reduce(
                        out=dummy_v.broadcast_to([128, D]),
                        accum_out=dot[:, g:g + 1],
                        in0=h_g, in1=q16[:], scale=1.0, bias=0.0,
                    )

                # rstd = 1/sqrt(ss/D + eps); logits = dot * rstd; e = exp
                sd = stats.tile([128, groups], F32, tag="sd")
                nc.scalar.activation(
                    out=sd[:], in_=ss[:],
                    func=mybir.ActivationFunctionType.Sqrt,
                    bias=epst[:], scale=1.0 / D,
                )
                rstd = stats.tile([128, groups], F32, tag="rstd")
                nc.vector.reciprocal(out=rstd[:], in_=sd[:])
                logit = stats.tile([128, groups], F32, tag="logit")
                nc.vector.tensor_mul(logit[:], dot[:], rstd[:])
                e = stats.tile([128, groups], F32, tag="e")
                nc.scalar.activation(
                    out=e[:], in_=logit[:],
                    func=mybir.ActivationFunctionType.Exp,
                )

                # sumexp over s: [8t', G] = mask^T @ e (exact fp32 matmul)
                se_ps = ps_stats.tile([TG, groups], F32, tag="se")
                nc.tensor.matmul(out=se_ps[:], lhsT=mask[:], rhs=e[:],
                                 start=True, stop=True)
                rse = stats.tile([TG, groups], F32, tag="rse")
                nc.vector.reciprocal(out=rse[:], in_=se_ps[:])
                # broadcast rse back to (s,t') partitions: maskT^T @ rse
                rseb_ps = ps_stats.tile([128, groups], F32, tag="rseb")
                nc.tensor.matmul(out=rseb_ps[:], lhsT=maskT[:], rhs=rse[:],
                                 start=True, stop=True)
                rseb = stats.tile([128, groups], F32, tag="rsebs")
                nc.vector.tensor_copy(out=rseb[:], in_=rseb_ps[:])

                # depth mix: accumulate 16 masked-weight fp16 matmuls per
                # D chunk; w2 built on VectorE
                m_ps = [ps_mix.tile([TG * groups, 512], F32, tag="m", name=f"m{c}")
                        for c in range(2)]
                for g in range(groups):
                    w2 = w2pool.tile([128, TG * groups], F16, tag="w2")
                    nc.vector.tensor_scalar(
                        out=w2[:],
                        in0=(maskF[:, g, :] if groups == GROUPS
                             else maskF8[:, g, :]),
                        scalar1=e[:, g:g + 1],
                        scalar2=rseb[:, g:g + 1],
                        op0=mybir.AluOpType.mult,
                        op1=mybir.AluOpType.mult,
                    )
                    for c in range(2):
                        nc.tensor.matmul(
                            out=m_ps[c][:],
                            lhsT=w2[:],
                            rhs=hslice(g)[:, c * 512:(c + 1) * 512],
                            start=(g == 0),
                            stop=(g == groups - 1),
                        )

                # PSUM -> SBUF output copies on ScalarE (activation Copy)
                # to take them off the saturated VectorE
                ot = outpool.tile([TG * groups, D], F16, tag="ot")
                for c in range(2):
                    nc.scalar.copy(out=ot[:, c * 512:(c + 1) * 512],
                                   in_=m_ps[c][:])
                nc.scalar.dma_start(out=out[t0:t0 + ts_k, :], in_=ot[:])

    nc.compile()
    return nc


_NC = None


def _get_program():
    global _NC
    if _NC is None:
        _NC = _build_program()
    return _NC


def _make_masks():
    # partition p = s*TG + t'; group slice g holds t_local = t'*GROUPS + g
    p = np.arange(128)
    mask = (p[:, None] % TG == np.arange(TG)[None, :]).astype(np.float32)
    maskF = np.zeros((128, GROUPS, 128), np.float16)
    for g in range(GROUPS):
        maskF[p, g, (p % TG) * GROUPS + g] = 1.0
    maskF8 = np.zeros((128, GROUPS // 2, 64), np.float16)
    for g in range(GROUPS // 2):
        maskF8[p, g, (p % TG) * (GROUPS // 2) + g] = 1.0
    return mask, np.ascontiguousarray(mask.T), maskF, maskF8


def kernel(history, query, rms_weight):
    history = np.asarray(history, dtype=np.float32)
    query = np.asarray(query, dtype=np.float32)
    rms_weight = np.asarray(rms_weight, dtype=np.float32)
    assert history.shape == (S, B, T, D), history.shape

    nc = _get_program()
    mask, maskT, maskF, maskF8 = _make_masks()

    in_maps = []
    for c in range(N_CORES):
        b, h = c // 2, c % 2
        shard = np.ascontiguousarray(history[:, b, h * TC:(h + 1) * TC, :])
        in_maps.append({
            "hist": shard,
            "query": query,
            "rms_weight": rms_weight,
            "mask": mask,
            "maskT": maskT,
            "maskF": maskF,
            "maskF8": maskF8,
        })

    res = bass_utils.run_bass_kernel_spmd(nc, in_maps, list(range(N_CORES)))

    out = np.empty((B, T, D), dtype=np.float32)
    for c in range(N_CORES):
        b, h = c // 2, c % 2
        out[b, h * TC:(h + 1) * TC, :] = res.results[c]["out"].astype(np.float32)
    return out



# revision 7
# speedup vs baseline: 1.0769x; 1.0769x over previous
"""DepthAttentionResidual Trainium2 kernel.

Computation (see reference):
    ms      = mean(history^2, axis=-1)                      # [S,B,T]
    logits  = dot(query*rms_weight, history) * rsqrt(ms+eps)
    w       = softmax(logits, axis=S)
    out     = sum_s w[s] * history[s]                        # [B,T,D]

Sharding: data-parallel over (B=4) x (T halves) = 8 cores. Each core gets
hist [S=16, Tc=1024, D=1024] (64 MiB) and produces out [1024, 1024].

Per-core layout: partition p = s*8 + t' (S=16 depths x 8 t-blocks), D on
the free axis. A supertile is 128 t; slice g holds t_local = t'*16 + g,
so one slice is [128 partitions, 1024].

Evolution of the bottleneck (HW-measured):
  * fp32->fp16 cast during the SWDGE DMA halves SBUF-side traffic; DMA
    paces at ~21us/supertile, engines must fit under that.
  * v1 (257us): ScalarE 16x(ACTIVATE Square 1147 + READ_ACC 279)ns and
    VectorE 17x1304ns 1x-mode reduces both ran ~28us/supertile.
  * v2: dots move to a custom DVE op SEG_SCAN_MUL_ANT (inclusive
    prefix-scan of in0*in1). The out AP broadcasts each segment to one
    [128,1] slot, so last-write-wins leaves the cumulative prefix at
    each segment end; a cheap shifted-subtract recovers per-group dots.
    One instruction covers 8 groups -> no per-group op overhead.
  * v2: sqrt+exp alternated ACT table sets (2x1283ns ACT_TABLE_LOAD per
    supertile). rstd = exp(-0.5*ln(ms+eps)) keeps everything in the
    natural_log_exp set -> one load total.
  * v2: mix weights built in one broadcast tensor_tensor (w2all =
    maskF * (e*rse_b)) instead of 16 tensor_scalars; PSUM->SBUF output
    copies split one-per-engine (PSUM->DRAM DMA is not allowed); the
    out DMA rides the idle Sync queue.
  * v3 (ENABLE_2X): hand-authored 2X_1PORT uop program for the scan op
    (pair-multiplies in blk0/blk1, pair-sum blk2, recurrence blk3, one
    fp32 write/cycle -> dst segment extent is 512, not 1024). Dot scans
    halve; n_vec_ss sumsq groups move to VectorE as h*h scans.
GpSimd carries NOTHING but the SWDGE dma_starts: any compute on the
in-order Pool queue starves the descriptor ring (+40-70us measured).
"""
import numpy as np

import concourse.bass as bass
import concourse.bacc as bacc
import concourse.tile as tile
from concourse import mybir
from concourse import bass_utils

import concourse.dve_ops as dve_ops_mod
from concourse.dve_ops import DveOp
from concourse.dve_spec import Spec, Bin, Src0, Src1, scan as dve_scan
from concourse.dve_spec import lower as dve_lower
from concourse.dve_uop import (
    AluOp as UAluOp,
    AluInp,
    DelayInp,
    DveOpSpec,
    InpSel,
    OutPath,
    OutSel,
    Trigger as UTrigger,
    UopConfig,
    UopDpConfig,
)

N_CORES = 8
S = 16
B = 4
T = 2048
D = 1024
EPS = 1e-5

TC = T // 2          # t positions per core
TG = 8               # t-blocks per partition set (S * TG = 128 partitions)
GROUPS = 16          # stat slices per supertile (one t per partition each)
J = 2                # consecutive t rows per DMA slice (read desc = J*4KiB)
TS = TG * GROUPS     # t per supertile = 128
N_SUPER = TC // TS   # supertiles per core = 8
F32 = mybir.dt.float32
F16 = mybir.dt.float16

# --- tuning flags ---------------------------------------------------------
ENABLE_2X = False     # use the hand-authored 2X_1PORT uop variant for scans
N_VEC_SS = 0          # sumsq groups (from the top) computed on VectorE scans
USE_LN_EXP = True     # rstd = exp(-0.5 ln(ms+eps)) instead of sqrt+recip
OUT_DMA = "sync"      # engine queue for the PSUM->HBM output DMA


# --- custom DVE op: segmented multiply-scan -------------------------------
SEG_SCAN_NAME = "SEG_SCAN_MUL_ANT"


def _build_2x_uops():
    """2X_1PORT program: two packed fp16 elems/cycle on each src port.
    blk0: m0 = src0_lo * src1_lo;  blk1: m1 = src0_hi * src1_hi (m0 parked
    on delay lane 4);  blk2: p = m0 + m1;  blk3: s = CURR + p (the scan
    recurrence);  blk4-7 bypass chain;  one fp32 WR0_LO write per cycle.
    Seed state writes the scan init (0) into blk3's flop."""
    def dp_chain(blks):
        for b in blks[4:]:
            b.pass_through_alu()
        return blks

    # steady
    st = UopConfig()
    st.enable_input(InpSel.SRC_0, 1)      # -> delay chain 0
    st.enable_input(InpSel.SRC_1, 2)      # -> delay chain 1
    st.enable_input(InpSel.SRC_0_HI, 3)   # -> delay chain 2
    st.enable_input(InpSel.SRC_1_HI, 4)   # -> delay chain 3
    st.require_inp0 = 1
    st.require_inp1 = 1
    st.trigger = (UTrigger.SRC_TENSOR_DONE, UTrigger.NONE, UTrigger.NONE)
    st.next_uop = (0, 0, 0)
    b = st.datapath_config
    b[0].enable_alu(UAluOp.MULTIPLY, AluInp.PREV_DELAY_0, AluInp.PREV_DELAY_1)
    b[0].pass_through_delay(2, 3)
    b[1].enable_alu(UAluOp.MULTIPLY, AluInp.PREV_DELAY_2, AluInp.PREV_DELAY_3)
    b[1].enable_delay_from_src(DelayInp.PREV_ALU_OUT, 4)   # park m0
    b[2].enable_alu(UAluOp.ADD, AluInp.PREV_ALU_OUT, AluInp.PREV_DELAY_4)
    b[3].enable_alu(UAluOp.ADD, AluInp.CURR_ALU_OUT, AluInp.PREV_ALU_OUT)
    dp_chain(b)
    st.enable_output(OutSel.ALU_OUT, OutPath.WR0_LO)

    # seed: one non-consuming cycle writing 0 into blk3's flop
    sd = UopConfig()
    sd.enable_input(InpSel.ZERO, 1)       # -> delay chain 0
    sd.trigger = (UTrigger.COUNT, UTrigger.NONE, UTrigger.NONE)
    sd.repeat_count = 1
    sd.next_uop = (1, 0, 0)
    c = sd.datapath_config
    c[0].pass_through_delay(0)
    c[1].pass_through_delay(0)
    c[2].pass_through_delay(0)
    c[3].enable_alu(UAluOp.BYPASS, AluInp.PREV_DELAY_0)
    dp_chain(c)
    return [sd, st]


def _register_seg_scan():
    if SEG_SCAN_NAME in dve_ops_mod._SUB_OPCODE_FOR_NAME:
        for op in dve_ops_mod.OPS:
            if op.name == SEG_SCAN_NAME:
                return op
    body = dve_scan(UAluOp.ADD, Bin(UAluOp.MULTIPLY, Src0, Src1))

    def _ref(in0, in1, s0, s1, imm2):
        prod = in0.astype(np.float32) * np.asarray(in1, np.float32)
        return np.cumsum(prod.reshape(prod.shape[0], -1), axis=-1)

    spec = Spec(body=body, reference=_ref)
    row = dve_ops_mod._CUSTOM_DVE_ROW_BASE + len(dve_ops_mod.OPS)
    assert row < 0x20
    uops = dve_lower(spec, ver="v3")
    dspec = DveOpSpec(name=SEG_SCAN_NAME, opcode=row, uops=uops, rd1_en=True)
    if ENABLE_2X:
        dspec.uops_2x = _build_2x_uops()
        dspec.perf_max = 1
    sha = dspec.sha("v3")
    op = DveOp(name=SEG_SCAN_NAME, spec=spec, subdim=False,
               uops_sha={"v3": sha})
    dve_ops_mod.OPS.append(op)
    dve_ops_mod.CUSTOM_DVE_SPECS[SEG_SCAN_NAME] = spec
    dve_ops_mod._SUB_OPCODE_FOR_NAME[SEG_SCAN_NAME] = row
    dve_ops_mod._COMPILE_CACHE[(SEG_SCAN_NAME, "v3")] = dspec
    return op


SEG_SCAN_OP = _register_seg_scan()


def _emit_scan(nc, out_tile, ngroups, in0, in1):
    """One SEG_SCAN_MUL instruction: prefix-scan of in0*in1 over the whole
    stream; out_tile [128, ngroups, 1] catches the cumulative prefix at
    each 1024-element segment boundary via a stride-0 broadcast dst."""
    ext = (D // 2) if ENABLE_2X else D
    inst = nc.vector._custom_dve(
        SEG_SCAN_OP,
        out=out_tile.broadcast_to([128, ngroups, ext]),
        in0=in0,
        in1=in1,
    )
    if ENABLE_2X:
        inst.ins.perf_max = 1
    return inst


def _build_program():
    nc = bacc.Bacc("TRN2", target_bir_lowering=False, debug=False,
                   enable_asserts=True, num_devices=N_CORES)

    hist = nc.dram_tensor("hist", [S, TC, D], F32, kind="ExternalInput").ap()
    query = nc.dram_tensor("query", [D], F32, kind="ExternalInput").ap()
    rmsw = nc.dram_tensor("rms_weight", [D], F32, kind="ExternalInput").ap()
    mask_d = nc.dram_tensor("mask", [128, TG], F32, kind="ExternalInput").ap()
    maskt_d = nc.dram_tensor("maskT", [TG, 128], F32, kind="ExternalInput").ap()
    maskf_d = nc.dram_tensor("maskF", [128, GROUPS, 128], F16,
                             kind="ExternalInput").ap()
    maskf8_d = nc.dram_tensor("maskF8", [128, GROUPS // 2, 128 // 2], F16,
                              kind="ExternalInput").ap()
    out = nc.dram_tensor("out", [TC, D], F16, kind="ExternalOutput").ap()

    out_eng = {"sync": "sync", "scalar": "scalar"}[OUT_DMA]

    with tile.TileContext(nc) as tc:
        with (
            tc.tile_pool(name="singles", bufs=1) as singles,
            tc.tile_pool(name="hsup", bufs=2) as hpool,
            tc.tile_pool(name="stats", bufs=2) as stats,
            tc.tile_pool(name="w2", bufs=3) as w2pool,
            tc.tile_pool(name="ps_stats", bufs=2, space="PSUM") as ps_stats,
            tc.tile_pool(name="ps_mix", bufs=4, space="PSUM") as ps_mix,
        ):
            # ---- constants ------------------------------------------------
            qf = singles.tile([128, D], F32)
            wf = singles.tile([128, D], F32)
            q16 = singles.tile([128, 1, D], F16)
            mask = singles.tile([128, TG], F32)
            maskT = singles.tile([TG, 128], F32)
            maskF = singles.tile([128, GROUPS, 128], F16)
            maskF8 = singles.tile([128, GROUPS // 2, 128 // 2], F16)
            epst = singles.tile([128, 1], F32)
            dummy_a = singles.tile([128, 1], F32)

            def emit_init():
                # constants ride the ScalarE HWDGE ring so the SWDGE
                # (gpsimd) queue carries only the history stream
                nc.scalar.dma_start(
                    out=qf[:],
                    in_=bass.AP(tensor=query.tensor, offset=0,
                                ap=[[0, 128], [1, D]]),
                )
                nc.scalar.dma_start(
                    out=wf[:],
                    in_=bass.AP(tensor=rmsw.tensor, offset=0,
                                ap=[[0, 128], [1, D]]),
                )
                nc.vector.tensor_mul(q16[:, 0, :], qf[:], wf[:])
                nc.scalar.dma_start(out=mask[:], in_=mask_d)
                nc.scalar.dma_start(out=maskT[:], in_=maskt_d)
                nc.scalar.dma_start(out=maskF[:], in_=maskf_d)
                nc.scalar.dma_start(out=maskF8[:], in_=maskf8_d)
                nc.vector.memset(epst[:], EPS)

            # ---- main loop over supertiles --------------------------------
            emit_init()
            schedule = [(k * TS, GROUPS) for k in range(N_SUPER - 1)]
            schedule += [((N_SUPER - 1) * TS, GROUPS // 2),
                         ((N_SUPER - 1) * TS + TS // 2, GROUPS // 2)]
            for k, (t0, groups) in enumerate(schedule):
                ts_k = TG * groups
                ndma = groups // J

                half = max(ndma // 2, 1)
                hsupA = hpool.tile([128, half, J, D], F16, tag="hsupA",
                                   name="hsupA", bufs=5)
                hsupB = (hpool.tile([128, ndma - half, J, D], F16,
                                    tag="hsupB", name="hsupB", bufs=5)
                         if ndma > half else None)

                def hslice(g, hsupA=hsupA, hsupB=hsupB, half=half):
                    gd, j = g // J, g % J
                    tile_ = hsupA if gd < half else hsupB
                    return tile_[:, gd - half if gd >= half else gd, j, :]

                srcv = hist[:, t0:t0 + ts_k, :].rearrange(
                    "s (t gd j) d -> s t gd (j d)", t=TG, gd=ndma)
                for gd in range(ndma):
                    tile_ = hsupA if gd < half else hsupB
                    nc.gpsimd.dma_start(
                        out=tile_[:, gd - half if gd >= half else gd, :, :]
                        .rearrange("p j d -> p (j d)"),
                        in_=srcv[:, :, gd, :])

                gA = half * J              # groups in hsupA
                gB = groups - gA           # groups in hsupB

                # ---- dots via prefix-scan (VectorE) -----------------------
                PA = stats.tile([128, gA, 1], F32, tag="PA")
                _emit_scan(nc, PA, gA,
                           hsupA[:].rearrange("p a j d -> p (a j) d"),
                           q16.broadcast_to([128, gA, D]))
                if hsupB is not None:
                    PB = stats.tile([128, gB, 1], F32, tag="PB")
                    _emit_scan(nc, PB, gB,
                               hsupB[:].rearrange("p a j d -> p (a j) d"),
                               q16.broadcast_to([128, gB, D]))

                dot = stats.tile([128, groups], F32, tag="dot")
                nc.vector.tensor_copy(out=dot[:, 0:1], in_=PA[:, 0, :])
                nc.vector.tensor_sub(dot[:, 1:gA], PA[:, 1:gA, 0],
                                     PA[:, 0:gA - 1, 0])
                if hsupB is not None:
                    nc.vector.tensor_copy(out=dot[:, gA:gA + 1],
                                          in_=PB[:, 0, :])
                    nc.vector.tensor_sub(dot[:, gA + 1:groups],
                                         PB[:, 1:gB, 0], PB[:, 0:gB - 1, 0])

                # ---- sumsq: ScalarE activation + optional VectorE scans ---
                n_vec = min(N_VEC_SS, gB) if hsupB is not None else 0
                n_act = groups - n_vec
                ss = stats.tile([128, groups], F32, tag="ss")
                for g in range(n_act):
                    nc.scalar.activation(
                        out=dummy_a.broadcast_to([128, D]),
                        in_=hslice(g),
                        func=mybir.ActivationFunctionType.Square,
                        accum_out=ss[:, g:g + 1],
                    )
                if n_vec:
                    # whole-of-tail scans over hsupB's last n_vec groups
                    a0 = (gB - n_vec) // J
                    PS = stats.tile([128, n_vec, 1], F32, tag="PS")
                    src = hsupB[:, a0:, :, :].rearrange("p a j d -> p (a j) d")
                    _emit_scan(nc, PS, n_vec, src, src)
                    g0 = groups - n_vec
                    nc.vector.tensor_copy(out=ss[:, g0:g0 + 1],
                                          in_=PS[:, 0, :])
                    nc.vector.tensor_sub(ss[:, g0 + 1:groups],
                                         PS[:, 1:n_vec, 0],
                                         PS[:, 0:n_vec - 1, 0])

                # ---- softmax over s ---------------------------------------
                e = stats.tile([128, groups], F32, tag="e")
                if USE_LN_EXP:
                    # rstd = exp(-0.5 ln(ms+eps)); ln+exp+exp live in one
                    # ACT table set (natural_log_exp_and_others)
                    lt = stats.tile([128, groups], F32, tag="lt")
                    nc.scalar.activation(
                        out=lt[:], in_=ss[:],
                        func=mybir.ActivationFunctionType.Ln,
                        bias=epst[:], scale=1.0 / D,
                    )
                    rstd = stats.tile([128, groups], F32, tag="rstd")
                    nc.scalar.activation(
                        out=rstd[:], in_=lt[:],
                        func=mybir.ActivationFunctionType.Exp,
                        scale=-0.5,
                    )
                else:
                    sd = stats.tile([128, groups], F32, tag="sd")
                    nc.scalar.activation(
                        out=sd[:], in_=ss[:],
                        func=mybir.ActivationFunctionType.Sqrt,
                        bias=epst[:], scale=1.0 / D,
                    )
                    rstd = stats.tile([128, groups], F32, tag="rstd")
                    nc.vector.reciprocal(out=rstd[:], in_=sd[:])
                logit = stats.tile([128, groups], F32, tag="logit")
                nc.vector.tensor_mul(logit[:], dot[:], rstd[:])
                nc.scalar.activation(
                    out=e[:], in_=logit[:],
                    func=mybir.ActivationFunctionType.Exp,
                )

                # sumexp over s: [8t', G] = mask^T @ e (exact fp32 matmul)
                se_ps = ps_stats.tile([TG, groups], F32, tag="se")
                nc.tensor.matmul(out=se_ps[:], lhsT=mask[:], rhs=e[:],
                                 start=True, stop=True)
                rse = stats.tile([TG, groups], F32, tag="rse")
                nc.vector.reciprocal(out=rse[:], in_=se_ps[:])
                # broadcast rse back to (s,t') partitions: maskT^T @ rse
                rseb_ps = ps_stats.tile([128, groups], F32, tag="rseb")
                nc.tensor.matmul(out=rseb_ps[:], lhsT=maskT[:], rhs=rse[:],
                                 start=True, stop=True)

                # ---- mix weights: one broadcast tensor_tensor -------------
                ww16 = stats.tile([128, groups, 1], F16, tag="ww16")
                nc.vector.tensor_mul(ww16[:, :, 0], e[:], rseb_ps[:])
                w2all = w2pool.tile([128, groups, ts_k], F16, tag="w2all")
                nc.vector.tensor_mul(
                    w2all[:],
                    (maskF[:, :, :ts_k] if groups == GROUPS
                     else maskF8[:, :, :ts_k]),
                    ww16.broadcast_to([128, groups, ts_k]),
                )

                # ---- depth mix + output -----------------------------------
                m_ps = [ps_mix.tile([TG * groups, 512], F32, tag="m",
                                    name=f"m{c}") for c in range(2)]
                for g in range(groups):
                    for c in range(2):
                        nc.tensor.matmul(
                            out=m_ps[c][:],
                            lhsT=w2all[:, g, :],
                            rhs=hslice(g)[:, c * 512:(c + 1) * 512],
                            start=(g == 0),
                            stop=(g == groups - 1),
                        )
                # PSUM -> SBUF copies, one per engine to split the load
                ot = w2pool.tile([TG * groups, D], F16, tag="ot")
                nc.scalar.copy(out=ot[:, 0:512], in_=m_ps[0][:])
                nc.vector.tensor_copy(out=ot[:, 512:1024], in_=m_ps[1][:])
                getattr(nc, out_eng).dma_start(
                    out=out[t0:t0 + ts_k, :], in_=ot[:])

    nc.compile()
    return nc


_NC = None


def _get_program():
    global _NC
    if _NC is None:
        _NC = _build_program()
    return _NC


def _make_masks():
    # partition p = s*TG + t'; group slice g holds t_local = t'*GROUPS + g
    p = np.arange(128)
    mask = (p[:, None] % TG == np.arange(TG)[None, :]).astype(np.float32)
    maskF = np.zeros((128, GROUPS, 128), np.float16)
    for g in range(GROUPS):
        maskF[p, g, (p % TG) * GROUPS + g] = 1.0
    maskF8 = np.zeros((128, GROUPS // 2, 64), np.float16)
    for g in range(GROUPS // 2):
        maskF8[p, g, (p % TG) * (GROUPS // 2) + g] = 1.0
    return mask, np.ascontiguousarray(mask.T), maskF, maskF8


def kernel(history, query, rms_weight):
    history = np.asarray(history, dtype=np.float32)
    query = np.asarray(query, dtype=np.float32)
    rms_weight = np.asarray(rms_weight, dtype=np.float32)
    assert history.shape == (S, B, T, D), history.shape

    nc = _get_program()
    mask, maskT, maskF, maskF8 = _make_masks()

    in_maps = []
    for c in range(N_CORES):
        b, h = c // 2, c % 2
        shard = np.ascontiguousarray(history[:, b, h * TC:(h + 1) * TC, :])
        in_maps.append({
            "hist": shard,
            "query": query,
            "rms_weight": rms_weight,
            "mask": mask,
            "maskT": maskT,
            "maskF": maskF,
            "maskF8": maskF8,
        })

    res = bass_utils.run_bass_kernel_spmd(nc, in_maps, list(range(N_CORES)))

    out = np.empty((B, T, D), dtype=np.float32)
    for c in range(N_CORES):
        b, h = c // 2, c % 2
        out[b, h * TC:(h + 1) * TC, :] = res.results[c]["out"].astype(np.float32)
    return out
